# revision 26
# baseline (speedup 1.0000x reference)
"""Multi-head self-attention on 8 Trainium2 NeuronCores.

Problem: X[2,2048,2048] -> MHA(16 heads, head_dim 128) -> [2,2048,2048].

Sharding: core c in 0..7 handles batch b = c // 4 and head-group g = c % 4
(4 heads = 512 hidden columns per core).  Each core computes
    Q^T,K^T,V  (its 512-column slice of the QKV projections)
    per-head attention (softmax without max-subtraction; inputs are bounded)
    partial output projection  out_heads @ Wo[512-slice, :]  -> [2048, 2048]
The 4 partial projections per batch are summed on the host (the tensor-
parallel "all-reduce" is done in numpy) and the output bias is added there.
Partial projections travel as bf16 (their sum is recovered in f32 on host).

Schedule (single pass; PE busy is within ~6% of the matmul streaming floor):
  phase A   stream-start: K-proj for half of S is accumulated k-outer across
            all 8 PSUM banks while the wk/xt DMA chunks land, so the PE works
            through the initial HBM load instead of waiting for it.
            Then K(rest), Q (everything resident).
  phase A/B seam: V-projection groups interleave with chunk-0 scores+exp so
            the ACT engine warms up while the PE finishes phase A.
  phase B   per q-chunk c: scores -> exp(ACT) -> denom (pairwise DVE tree
            into a scratch tile, decoupled from AV so it finishes right
            after the last exp + one ones-matmul to broadcast over
            partitions) -> AV -> reciprocal scale.  Each head's OM follows
            its AV immediately (denom already resolved), and the output
            projection of chunk c-1 fills the PE gaps while ACT works
            through chunk c's exps.  The last head's OM carries into the
            next chunk so its exp tail never blocks the PE.
  phase C   final chunk's projection drains at the end (bf16 y DMA), with
            drains alternating between the DVE and ACT queues.
"""

import math
import sys

import numpy as np

sys.path.insert(0, "/opt/trn_rl_repo")

import ml_dtypes  # noqa: E402

import concourse.bass as bass  # noqa: E402
import concourse.mybir as mybir  # noqa: E402
import concourse.tile as tile  # noqa: E402
from concourse import bacc  # noqa: E402

B, S, H = 2, 2048, 2048
HEADS, D = 16, 128
NC = 8
GROUPS = 4            # cores per batch (head-group parallel)
CW = H // GROUPS      # 512 hidden columns per core (4 heads)
HG = CW // D          # 4 heads per core
P = 128               # partitions
FN = 512              # matmul free-dim / psum bank (f32)
KT = H // P           # 16 contraction tiles for the projections
SQ = S // FN          # 4 query chunks of 512
SK = S // P           # 16 key tiles of 128

BF16 = mybir.dt.bfloat16
F32 = mybir.dt.float32

_CACHE = {}


def _build_nc():
    nc = bacc.Bacc()
    xt = nc.dram_tensor("xt", [H, S], BF16, kind="ExternalInput")
    wq = nc.dram_tensor("wq", [H, CW], BF16, kind="ExternalInput")
    wk = nc.dram_tensor("wk", [H, CW], BF16, kind="ExternalInput")
    wv = nc.dram_tensor("wv", [H, CW], BF16, kind="ExternalInput")
    wo = nc.dram_tensor("wo", [CW, H], BF16, kind="ExternalInput")
    bqk = nc.dram_tensor("bqk", [P, 2 * HG], F32, kind="ExternalInput")
    bv = nc.dram_tensor("bv", [CW], F32, kind="ExternalInput")
    y = nc.dram_tensor("y", [S, H], BF16, kind="ExternalOutput")

    with tile.TileContext(nc) as tc:
        _emit(nc, tc, xt[:], wq[:], wk[:], wv[:], wo[:], bqk[:], bv[:], y[:])
    nc.finalize()
    return nc


def _emit(nc, tc, xt, wq, wk, wv, wo, bqk, bv, y):
    from contextlib import ExitStack

    with ExitStack() as ctx:
        # ---- long-lived pools (left SBUF stack) --------------------------
        consts = ctx.enter_context(tc.tile_pool(name="consts", bufs=1, side="left"))
        qkv = ctx.enter_context(tc.tile_pool(name="qkv", bufs=1, side="left"))
        # 8 PSUM banks, shared by every phase (no pool churn):
        #   ps_s  2 x [P,2,FN] = 4 banks   scores / stream
        #   ps_av 2 x [P,FN]   = 2 banks   AV accumulations / stream
        #   ps_sh 2 x [P,FN]   = 2 banks   projections + softmax denoms / stream
        ps_s = ctx.enter_context(tc.tile_pool(name="ps_s", bufs=2, space="PSUM"))
        ps_av = ctx.enter_context(tc.tile_pool(name="ps_av", bufs=2, space="PSUM"))
        ps_sh = ctx.enter_context(tc.tile_pool(name="ps_sh", bufs=2, space="PSUM"))

        ones_s = consts.tile([P, P], BF16)
        nc.vector.memset(ones_s, 1.0)
        bqk_s = consts.tile([P, 2 * HG], F32)   # [:, :HG]=bq, [:, HG:]=bk
        bv_row = consts.tile([P, CW], F32)

        qt = qkv.tile([P, HG, S], BF16)     # Q^T: [d-part, head, S]
        kt_sb = qkv.tile([P, HG, S], BF16)
        v_sb = qkv.tile([P, SK, CW], BF16)  # V: [S-part, S-tile, 4 heads*d]

        def pss_tile():
            return ps_s.tile([P, 2, FN], F32, tag="s", name="pss")

        def pav_tile():
            return ps_av.tile([P, FN], F32, tag="av", name="pav")

        def psh_tile():
            return ps_sh.tile([P, FN], F32, tag="sh", name="psh")

        # ---- phase A: bulk loads (right SBUF stack) + projections --------
        # xtw is released manually after the V-projection so the wo pool can
        # take its place while the left-stack pools stay live.
        xtw = tc.alloc_tile_pool(name="xtw", bufs=1, side="right")
        xts = xtw.tile([P, KT, S], BF16, name="xts")
        wv_s = xtw.tile([P, KT, CW], BF16, name="wv_s")
        with tc.tile_pool(name="wqk", bufs=1, side="right") as wqk:
            wq_s = wqk.tile([P, KT, CW], BF16)
            wk_s = wqk.tile([P, KT, CW], BF16)

            # bqk rides the idle scalar hardware DMA queue; the bulk load is
            # one sync-queue stream in strict need-order: the k-outer stream
            # rounds only touch xt columns 0:1024 (c=0,1), so those halves
            # interleave with wk and the 1024:2048 halves follow - the stream
            # phase becomes PE-paced instead of DMA-paced.
            nc.scalar.dma_start(bqk_s, bqk)
            xt_r = xt.rearrange("(a p) s -> p a s", p=P)
            wq_r = wq.rearrange("(a p) m -> p a m", p=P)
            wk_r = wk.rearrange("(a p) m -> p a m", p=P)
            wv_r = wv.rearrange("(a p) m -> p a m", p=P)
            HS = S // 2
            for k in range(KT):
                nc.sync.dma_start(wk_s[:, k], wk_r[:, k])
                nc.sync.dma_start(xts[:, k, :HS], xt_r[:, k, :HS])
            # everything past the stream phase is consumed whole, so issue it
            # as three big DMAs: the sync queue was issue-bound with 48 chunk
            # DMAs (~0.65us of queue time each)
            nc.sync.dma_start(xts[:, :, HS:], xt_r[:, :, HS:])
            nc.sync.dma_start(wq_s, wq_r)
            nc.sync.dma_start(wv_s, wv_r)
            nc.sync.dma_start(bv_row, bv[None, :].to_broadcast([P, CW]))

            # K-proj for c=0,1 streamed k-outer across all 8 banks: the k-th
            # round starts as soon as DMA chunk k lands.
            sgroups = [(m, c) for m in range(HG) for c in range(2)]
            t0, t1 = pss_tile(), pss_tile()
            stiles = [t0[:, 0], t0[:, 1], t1[:, 0], t1[:, 1],
                      pav_tile(), pav_tile(), psh_tile(), psh_tile()]
            for k in range(KT):
                # last round runs in drain order so the first-drained banks
                # finish their accumulation (and free up) soonest
                order = (6, 7, 0, 1, 2, 3, 4, 5) if k == KT - 1 else range(8)
                for g in order:
                    m, c = sgroups[g]
                    nc.tensor.matmul(
                        stiles[g],
                        wk_s[:, k, m * P:(m + 1) * P],
                        xts[:, k, c * FN:(c + 1) * FN],
                        start=(k == 0),
                        stop=(k == KT - 1),
                    )
            # drain the ps_sh tiles first: the next projection groups allocate
            # from that ring and would otherwise wait for the whole drain run
            for g in (6, 7, 0, 1, 2, 3, 4, 5):
                m, c = sgroups[g]
                nc.vector.tensor_scalar_add(
                    kt_sb[:, m, c * FN:(c + 1) * FN], stiles[g],
                    bqk_s[:, HG + m:HG + m + 1])

            # K-proj c=2,3 and all of Q: classic k-inner accumulation groups.
            for m in range(HG):
                for c in (2, 3):
                    ps = psh_tile()
                    for k in range(KT):
                        nc.tensor.matmul(
                            ps, wk_s[:, k, m * P:(m + 1) * P],
                            xts[:, k, c * FN:(c + 1) * FN],
                            start=(k == 0), stop=(k == KT - 1))
                    nc.vector.tensor_scalar_add(
                        kt_sb[:, m, c * FN:(c + 1) * FN], ps,
                        bqk_s[:, HG + m:HG + m + 1])
            for m in range(HG):
                for c in range(SQ):
                    ps = psh_tile()
                    for k in range(KT):
                        nc.tensor.matmul(
                            ps, wq_s[:, k, m * P:(m + 1) * P],
                            xts[:, k, c * FN:(c + 1) * FN],
                            start=(k == 0), stop=(k == KT - 1))
                    nc.vector.tensor_scalar_add(
                        qt[:, m, c * FN:(c + 1) * FN], ps,
                        bqk_s[:, m:m + 1])
        # wqk released (right stack) — et/outt below fit alongside xts+wv.

        # ---- phase B pools (left stack, live to the end) -----------------
        et_pool = ctx.enter_context(tc.tile_pool(name="expt", bufs=3, side="left"))
        rden_pool = ctx.enter_context(tc.tile_pool(name="rden", bufs=2, side="left"))
        ystage = ctx.enter_context(tc.tile_pool(name="ystage", bufs=3, side="left"))
        op = ctx.enter_context(tc.tile_pool(name="outt", bufs=1, side="left"))
        outt = op.tile([P, HG, S], BF16)    # out_heads^T: [d-part, head, S]

        def v_group(m):
            ps = psh_tile()
            for k in range(KT):
                nc.tensor.matmul(
                    ps, xts[:, k, m * P:(m + 1) * P], wv_s[:, k, :],
                    start=(k == 0), stop=(k == KT - 1))
            nc.vector.tensor_add(v_sb[:, m, :], ps, bv_row)

        def s_head(c, h):
            """scores^T -> exp for one (q-chunk, head): 8 psum fills + 8 ACTs."""
            et = et_pool.tile([P, SK, FN], BF16, tag="et", name="et")
            for j in range(SK // 2):
                ps = pss_tile()
                for i in range(2):
                    nc.tensor.matmul(
                        ps[:, i],
                        kt_sb[:, h, (2 * j + i) * P:(2 * j + i + 1) * P],
                        qt[:, h, c * FN:(c + 1) * FN],
                        start=True, stop=True)
                nc.scalar.activation(
                    et[:, 2 * j:2 * j + 2, :].rearrange("p a q -> p (a q)"),
                    ps.rearrange("p a q -> p (a q)"),
                    mybir.ActivationFunctionType.Exp)
            return et

        def av_head(c, h, et):
            """AV accumulation (subtile deps let it chase the exp tail)."""
            ps_o = pav_tile()
            for k in range(SK):
                nc.tensor.matmul(
                    ps_o, v_sb[:, k, h * P:(h + 1) * P], et[:, k, :],
                    start=(k == 0), stop=(k == SK - 1))
            return ps_o

        def tree_head(et):
            """Pairwise denom tree into a scratch tile on DVE — independent
            of AV, so it runs as soon as the exps land."""
            sc = sc_pool.tile([P, 8, FN], BF16, tag="sc", name="sc")
            # the et[0:8] half reduces while the ACT engine still produces
            # tiles 8..15; only ~1.3us of adds remain after the last exp
            nc.vector.tensor_add(sc[:, 0:4, :], et[:, 0:4, :], et[:, 4:8, :])
            nc.vector.tensor_add(sc[:, 0:2, :], sc[:, 0:2, :], sc[:, 2:4, :])
            nc.vector.tensor_add(sc[:, 0, :], sc[:, 0, :], sc[:, 1, :])
            nc.vector.tensor_add(sc[:, 4:8, :], et[:, 8:12, :], et[:, 12:16, :])
            nc.vector.tensor_add(sc[:, 4:6, :], sc[:, 4:6, :], sc[:, 6:8, :])
            nc.vector.tensor_add(sc[:, 0, :], sc[:, 0, :], sc[:, 4, :])
            nc.vector.tensor_add(sc[:, 0, :], sc[:, 0, :], sc[:, 5, :])
            return sc

        def om_head(c, h, sc, ps_o):
            """ones-matmul broadcast of the row sums, reciprocal, scale."""
            ps_d = psh_tile()
            nc.tensor.matmul(ps_d, ones_s, sc[:, 0, :], start=True, stop=True)
            rden = rden_pool.tile([P, FN], F32, tag="r", name="rden")
            nc.vector.reciprocal_approx_fast(out=rden, in_=ps_d)
            nc.vector.tensor_mul(outt[:, h, c * FN:(c + 1) * FN], ps_o, rden)

        def proj_group(c, gi, drain=None, ps=None):
            m = 4 * c + gi // (H // FN)
            c2 = gi % (H // FN)
            if ps is None:
                ps = psh_tile()
            for k in range(HG):
                nc.tensor.matmul(
                    ps, outt[:, k, m * P:(m + 1) * P],
                    wo_s[:, k, c2 * FN:(c2 + 1) * FN],
                    start=(k == 0), stop=(k == HG - 1))
            yt = ystage.tile([P, FN], BF16, tag="y", name="yt")
            if drain == "scalar":
                nc.scalar.copy(yt, ps)   # ACT is idle during the final drain
            else:
                nc.vector.tensor_copy(yt, ps)
            nc.sync.dma_start(y[m * P:(m + 1) * P, c2 * FN:(c2 + 1) * FN], yt)

        # ---- V-projection interleaved with chunk-0 scores ----------------
        et_c = {}
        pso_c = {}
        for m in range(5):
            v_group(m)
        et_c[0] = s_head(0, 0)
        for m in range(5, 10):
            v_group(m)
        et_c[1] = s_head(0, 1)
        for m in range(10, 14):
            v_group(m)
        et_c[2] = s_head(0, 2)
        for m in range(14, 16):
            v_group(m)

        xtw.release()   # right stack empty: scratch + wo fit on the left
        sc_pool = ctx.enter_context(tc.tile_pool(name="scr", bufs=3, side="left"))
        wo_pool = ctx.enter_context(tc.tile_pool(name="wo", bufs=1, side="left"))
        wo_s = wo_pool.tile([P, HG, H], BF16)
        nc.sync.dma_start(wo_s, wo.rearrange("(a p) n -> p a n", p=P))

        # ---- chunk 0 (no projection fill available yet) ------------------
        sc_c = {0: tree_head(et_c[0]), 1: tree_head(et_c[1])}
        pso_c[0] = av_head(0, 0, et_c[0])
        om_head(0, 0, sc_c[0], pso_c[0])
        sc_c[2] = tree_head(et_c[2])
        et_c[3] = s_head(0, 3)
        pso_c[1] = av_head(0, 1, et_c[1])
        om_head(0, 1, sc_c[1], pso_c[1])
        pso_c[2] = av_head(0, 2, et_c[2])
        om_head(0, 2, sc_c[2], pso_c[2])
        pso_c[3] = av_head(0, 3, et_c[3])
        sc_c[3] = tree_head(et_c[3])
        carry = (0, 3, sc_c[3], pso_c[3])   # OM deferred into the next chunk

        # ---- chunks 1..3: scores/AV with proj(c-1) as PE fill ------------
        for c in range(1, SQ):
            et_c = {}
            pso_c = {}
            sc_c = {}
            et_c[0] = s_head(c, 0)
            om_head(*carry)
            for gi in range(8):
                proj_group(c - 1, gi)
            sc_c[0] = tree_head(et_c[0])
            et_c[1] = s_head(c, 1)
            for gi in range(8, 16):
                proj_group(c - 1, gi)
            sc_c[1] = tree_head(et_c[1])
            et_c[2] = s_head(c, 2)
            pso_c[0] = av_head(c, 0, et_c[0])
            om_head(c, 0, sc_c[0], pso_c[0])
            sc_c[2] = tree_head(et_c[2])
            et_c[3] = s_head(c, 3)
            pso_c[1] = av_head(c, 1, et_c[1])
            om_head(c, 1, sc_c[1], pso_c[1])
            pso_c[2] = av_head(c, 2, et_c[2])
            om_head(c, 2, sc_c[2], pso_c[2])
            pso_c[3] = av_head(c, 3, et_c[3])
            sc_c[3] = tree_head(et_c[3])
            carry = (c, 3, sc_c[3], pso_c[3])

        # ---- final chunk's projection drains last ------------------------
        # scores/AV psum rings are dead here: rotate all 8 bank-slots so the
        # drains (alternating DVE/ACT, both busy with kernel-tail work) never
        # gate the PE on the usual 2-slot ring
        om_head(*carry)
        t0, t1 = pss_tile(), pss_tile()
        slots = [psh_tile(), psh_tile(), pav_tile(), pav_tile(),
                 t0[:, 0], t0[:, 1], t1[:, 0], t1[:, 1]]
        for gi in range(16):
            proj_group(SQ - 1, gi, drain="scalar", ps=slots[gi % 8])


def _get_nc():
    if "nc" not in _CACHE:
        _CACHE["nc"] = _build_nc()
    return _CACHE["nc"]


def make_in_maps(X, Wq, bq, Wk, bk, Wv, bv, Wo, bo):
    bf16 = ml_dtypes.bfloat16
    scale = 1.0 / math.sqrt(D)
    X = np.asarray(X, dtype=np.float32)
    xt_b = [np.ascontiguousarray(X[b].T).astype(bf16) for b in range(B)]
    Wq = np.asarray(Wq, dtype=np.float32) * scale
    Wk = np.asarray(Wk, dtype=np.float32)
    Wv = np.asarray(Wv, dtype=np.float32)
    Wo = np.asarray(Wo, dtype=np.float32)
    bq = np.asarray(bq, dtype=np.float32) * scale
    bk = np.asarray(bk, dtype=np.float32)
    bv = np.asarray(bv, dtype=np.float32)
    in_maps = []
    for c in range(NC):
        b, g = divmod(c, GROUPS)
        sl = slice(g * CW, (g + 1) * CW)
        # per-partition bias layout: bqk[p, m] = bq[m*128+p], cols 4..7 = bk
        bqk = np.concatenate(
            [bq[sl].reshape(HG, P).T, bk[sl].reshape(HG, P).T], axis=1)
        in_maps.append({
            "xt": xt_b[b],
            "wq": np.ascontiguousarray(Wq[:, sl]).astype(bf16),
            "wk": np.ascontiguousarray(Wk[:, sl]).astype(bf16),
            "wv": np.ascontiguousarray(Wv[:, sl]).astype(bf16),
            "wo": np.ascontiguousarray(Wo[sl, :]).astype(bf16),
            "bqk": np.ascontiguousarray(bqk, dtype=np.float32),
            "bv": np.ascontiguousarray(bv[sl]),
        })
    return in_maps


def gather_output(results, bo):
    bo = np.asarray(bo, dtype=np.float32)
    out = np.empty((B, S, H), np.float32)
    for b in range(B):
        acc = results[b * GROUPS]["y"].astype(np.float32)
        for g in range(1, GROUPS):
            acc += results[b * GROUPS + g]["y"].astype(np.float32)
        out[b] = acc + bo[None, :]
    return out


def kernel(X, Wq, bq, Wk, bk, Wv, bv, Wo, bo):
    from concourse.bass_utils import run_bass_kernel_spmd

    in_maps = make_in_maps(X, Wq, bq, Wk, bk, Wv, bv, Wo, bo)
    nc = _get_nc()
    res = run_bass_kernel_spmd(nc, in_maps, list(range(NC))).results
    return gather_output(res, bo)


# revision 29
# speedup vs baseline: 1.0199x; 1.0199x over previous
"""Multi-head self-attention on 8 Trainium2 NeuronCores.

Problem: X[2,2048,2048] -> MHA(16 heads, head_dim 128) -> [2,2048,2048].

Sharding: core c in 0..7 handles batch b = c // 4 and head-group g = c % 4
(4 heads = 512 hidden columns per core).  Each core computes
    Q^T,K^T,V  (its 512-column slice of the QKV projections)
    per-head attention (softmax without max-subtraction; inputs are bounded)
    partial output projection  out_heads @ Wo[512-slice, :]  -> [2048, 2048]
The 4 partial projections per batch are summed on the host (the tensor-
parallel "all-reduce" is done in numpy) and the output bias is added there.
Partial projections travel as bf16 (their sum is recovered in f32 on host).

Schedule (single pass; PE busy is within ~6% of the matmul streaming floor):
  phase A   stream-start: K-proj for half of S is accumulated k-outer across
            all 8 PSUM banks while the wk/xt DMA chunks land, so the PE works
            through the initial HBM load instead of waiting for it.
            Then K(rest), Q (everything resident).
  phase A/B seam: V-projection groups interleave with chunk-0 scores+exp so
            the ACT engine warms up while the PE finishes phase A.
  phase B   per q-chunk c: scores -> exp(ACT) -> denom (pairwise DVE tree
            into a scratch tile, decoupled from AV so it finishes right
            after the last exp + one ones-matmul to broadcast over
            partitions) -> AV -> reciprocal scale.  Each head's OM follows
            its AV immediately (denom already resolved), and the output
            projection of chunk c-1 fills the PE gaps while ACT works
            through chunk c's exps.  The last head's OM carries into the
            next chunk so its exp tail never blocks the PE.
  phase C   final chunk's projection drains at the end (bf16 y DMA), with
            drains alternating between the DVE and ACT queues.
"""

import math
import sys

import numpy as np

sys.path.insert(0, "/opt/trn_rl_repo")

import ml_dtypes  # noqa: E402

import concourse.bass as bass  # noqa: E402
import concourse.mybir as mybir  # noqa: E402
import concourse.tile as tile  # noqa: E402
from concourse import bacc  # noqa: E402

B, S, H = 2, 2048, 2048
HEADS, D = 16, 128
NC = 8
GROUPS = 4            # cores per batch (head-group parallel)
CW = H // GROUPS      # 512 hidden columns per core (4 heads)
HG = CW // D          # 4 heads per core
P = 128               # partitions
FN = 512              # matmul free-dim / psum bank (f32)
KT = H // P           # 16 contraction tiles for the projections
SQ = S // FN          # 4 query chunks of 512
SK = S // P           # 16 key tiles of 128

BF16 = mybir.dt.bfloat16
F32 = mybir.dt.float32

_CACHE = {}


def _build_nc():
    nc = bacc.Bacc()
    xt = nc.dram_tensor("xt", [H, S], BF16, kind="ExternalInput")
    wq = nc.dram_tensor("wq", [H, CW], BF16, kind="ExternalInput")
    wk = nc.dram_tensor("wk", [H, CW], BF16, kind="ExternalInput")
    wv = nc.dram_tensor("wv", [H, CW], BF16, kind="ExternalInput")
    wo = nc.dram_tensor("wo", [CW, H], BF16, kind="ExternalInput")
    bqk = nc.dram_tensor("bqk", [P, 2 * HG], F32, kind="ExternalInput")
    bv = nc.dram_tensor("bv", [CW], F32, kind="ExternalInput")
    y = nc.dram_tensor("y", [S, H], BF16, kind="ExternalOutput")

    with tile.TileContext(nc) as tc:
        _emit(nc, tc, xt[:], wq[:], wk[:], wv[:], wo[:], bqk[:], bv[:], y[:])
    nc.finalize()
    return nc


def _emit(nc, tc, xt, wq, wk, wv, wo, bqk, bv, y):
    from contextlib import ExitStack

    with ExitStack() as ctx:
        # ---- long-lived pools (left SBUF stack) --------------------------
        consts = ctx.enter_context(tc.tile_pool(name="consts", bufs=1, side="left"))
        qkv = ctx.enter_context(tc.tile_pool(name="qkv", bufs=1, side="left"))
        # 8 PSUM banks, shared by every phase (no pool churn):
        #   ps_s  2 x [P,2,FN] = 4 banks   scores / stream
        #   ps_av 2 x [P,FN]   = 2 banks   AV accumulations / stream
        #   ps_sh 2 x [P,FN]   = 2 banks   projections + softmax denoms / stream
        ps_s = ctx.enter_context(tc.tile_pool(name="ps_s", bufs=2, space="PSUM"))
        ps_av = ctx.enter_context(tc.tile_pool(name="ps_av", bufs=2, space="PSUM"))
        ps_sh = ctx.enter_context(tc.tile_pool(name="ps_sh", bufs=2, space="PSUM"))

        ones_s = consts.tile([P, P], BF16)
        nc.vector.memset(ones_s, 1.0)
        bqk_s = consts.tile([P, 2 * HG], F32)   # [:, :HG]=bq, [:, HG:]=bk
        bv_row = consts.tile([P, CW], F32)

        qt = qkv.tile([P, HG, S], BF16)     # Q^T: [d-part, head, S]
        kt_sb = qkv.tile([P, HG, S], BF16)
        v_sb = qkv.tile([P, SK, CW], BF16)  # V: [S-part, S-tile, 4 heads*d]

        def pss_tile():
            return ps_s.tile([P, 2, FN], F32, tag="s", name="pss")

        def pav_tile():
            return ps_av.tile([P, FN], F32, tag="av", name="pav")

        def psh_tile():
            return ps_sh.tile([P, FN], F32, tag="sh", name="psh")

        # ---- phase A: bulk loads (right SBUF stack) + projections --------
        # xtw is released manually after the V-projection so the wo pool can
        # take its place while the left-stack pools stay live.
        xtw = tc.alloc_tile_pool(name="xtw", bufs=1, side="right")
        xts = xtw.tile([P, KT, S], BF16, name="xts")
        wv_s = xtw.tile([P, KT, CW], BF16, name="wv_s")
        with tc.tile_pool(name="wqk", bufs=1, side="right") as wqk:
            wq_s = wqk.tile([P, KT, CW], BF16)
            wk_s = wqk.tile([P, KT, CW], BF16)

            # bqk rides the idle scalar hardware DMA queue; the bulk load is
            # one sync-queue stream in strict need-order: the k-outer stream
            # rounds only touch xt columns 0:1024 (c=0,1), so those halves
            # interleave with wk and the 1024:2048 halves follow - the stream
            # phase becomes PE-paced instead of DMA-paced.
            nc.scalar.dma_start(bqk_s, bqk)
            xt_r = xt.rearrange("(a p) s -> p a s", p=P)
            wq_r = wq.rearrange("(a p) m -> p a m", p=P)
            wk_r = wk.rearrange("(a p) m -> p a m", p=P)
            wv_r = wv.rearrange("(a p) m -> p a m", p=P)
            HS = S // 2
            for k in range(KT):
                nc.sync.dma_start(wk_s[:, k], wk_r[:, k])
                nc.sync.dma_start(xts[:, k, :HS], xt_r[:, k, :HS])
            for k in range(KT):
                nc.sync.dma_start(xts[:, k, HS:], xt_r[:, k, HS:])
            for k in range(KT):
                nc.sync.dma_start(wq_s[:, k], wq_r[:, k])
            for k in range(KT):
                nc.sync.dma_start(wv_s[:, k], wv_r[:, k])
            nc.sync.dma_start(bv_row, bv[None, :].to_broadcast([P, CW]))

            # K-proj for c=0,1 streamed k-outer across all 8 banks: the k-th
            # round starts as soon as DMA chunk k lands.
            sgroups = [(m, c) for m in range(HG) for c in range(2)]
            t0, t1 = pss_tile(), pss_tile()
            stiles = [t0[:, 0], t0[:, 1], t1[:, 0], t1[:, 1],
                      pav_tile(), pav_tile(), psh_tile(), psh_tile()]
            for k in range(KT):
                for g, (m, c) in enumerate(sgroups):
                    nc.tensor.matmul(
                        stiles[g],
                        wk_s[:, k, m * P:(m + 1) * P],
                        xts[:, k, c * FN:(c + 1) * FN],
                        start=(k == 0),
                        stop=(k == KT - 1),
                    )
            # drain the ps_sh tiles first: the next projection groups allocate
            # from that ring and would otherwise wait for the whole drain run
            for g in (6, 7, 0, 1, 2, 3, 4, 5):
                m, c = sgroups[g]
                nc.vector.tensor_scalar_add(
                    kt_sb[:, m, c * FN:(c + 1) * FN], stiles[g],
                    bqk_s[:, HG + m:HG + m + 1])

            # K-proj c=2,3 and all of Q: classic k-inner accumulation groups.
            for m in range(HG):
                for c in (2, 3):
                    ps = psh_tile()
                    for k in range(KT):
                        nc.tensor.matmul(
                            ps, wk_s[:, k, m * P:(m + 1) * P],
                            xts[:, k, c * FN:(c + 1) * FN],
                            start=(k == 0), stop=(k == KT - 1))
                    nc.vector.tensor_scalar_add(
                        kt_sb[:, m, c * FN:(c + 1) * FN], ps,
                        bqk_s[:, HG + m:HG + m + 1])
            for m in range(HG):
                for c in range(SQ):
                    ps = psh_tile()
                    for k in range(KT):
                        nc.tensor.matmul(
                            ps, wq_s[:, k, m * P:(m + 1) * P],
                            xts[:, k, c * FN:(c + 1) * FN],
                            start=(k == 0), stop=(k == KT - 1))
                    nc.vector.tensor_scalar_add(
                        qt[:, m, c * FN:(c + 1) * FN], ps,
                        bqk_s[:, m:m + 1])
        # wqk released (right stack) — et/outt below fit alongside xts+wv.

        # ---- phase B pools (left stack, live to the end) -----------------
        et_pool = ctx.enter_context(tc.tile_pool(name="expt", bufs=3, side="left"))
        rden_pool = ctx.enter_context(tc.tile_pool(name="rden", bufs=2, side="left"))
        ystage = ctx.enter_context(tc.tile_pool(name="ystage", bufs=3, side="left"))
        op = ctx.enter_context(tc.tile_pool(name="outt", bufs=1, side="left"))
        outt = op.tile([P, HG, S], BF16)    # out_heads^T: [d-part, head, S]

        def v_group(m):
            ps = psh_tile()
            for k in range(KT):
                nc.tensor.matmul(
                    ps, xts[:, k, m * P:(m + 1) * P], wv_s[:, k, :],
                    start=(k == 0), stop=(k == KT - 1))
            nc.vector.tensor_add(v_sb[:, m, :], ps, bv_row)

        def s_head(c, h):
            """scores^T -> exp for one (q-chunk, head): 8 psum fills + 8 ACTs."""
            et = et_pool.tile([P, SK, FN], BF16, tag="et", name="et")
            for j in range(SK // 2):
                ps = pss_tile()
                for i in range(2):
                    nc.tensor.matmul(
                        ps[:, i],
                        kt_sb[:, h, (2 * j + i) * P:(2 * j + i + 1) * P],
                        qt[:, h, c * FN:(c + 1) * FN],
                        start=True, stop=True)
                nc.scalar.activation(
                    et[:, 2 * j:2 * j + 2, :].rearrange("p a q -> p (a q)"),
                    ps.rearrange("p a q -> p (a q)"),
                    mybir.ActivationFunctionType.Exp)
            return et

        def av_head(c, h, et):
            """AV accumulation (subtile deps let it chase the exp tail)."""
            ps_o = pav_tile()
            for k in range(SK):
                nc.tensor.matmul(
                    ps_o, v_sb[:, k, h * P:(h + 1) * P], et[:, k, :],
                    start=(k == 0), stop=(k == SK - 1))
            return ps_o

        def tree_head(et):
            """Pairwise denom tree into a scratch tile on DVE — independent
            of AV, so it runs as soon as the exps land."""
            sc = sc_pool.tile([P, 8, FN], BF16, tag="sc", name="sc")
            # the et[0:8] half reduces while the ACT engine still produces
            # tiles 8..15; only ~1.3us of adds remain after the last exp
            nc.vector.tensor_add(sc[:, 0:4, :], et[:, 0:4, :], et[:, 4:8, :])
            nc.vector.tensor_add(sc[:, 0:2, :], sc[:, 0:2, :], sc[:, 2:4, :])
            nc.vector.tensor_add(sc[:, 0, :], sc[:, 0, :], sc[:, 1, :])
            nc.vector.tensor_add(sc[:, 4:8, :], et[:, 8:12, :], et[:, 12:16, :])
            nc.vector.tensor_add(sc[:, 4:6, :], sc[:, 4:6, :], sc[:, 6:8, :])
            nc.vector.tensor_add(sc[:, 0, :], sc[:, 0, :], sc[:, 4, :])
            nc.vector.tensor_add(sc[:, 0, :], sc[:, 0, :], sc[:, 5, :])
            return sc

        def om_head(c, h, sc, ps_o):
            """ones-matmul broadcast of the row sums, reciprocal, scale."""
            ps_d = psh_tile()
            nc.tensor.matmul(ps_d, ones_s, sc[:, 0, :], start=True, stop=True)
            rden = rden_pool.tile([P, FN], F32, tag="r", name="rden")
            nc.vector.reciprocal_approx_fast(out=rden, in_=ps_d)
            nc.vector.tensor_mul(outt[:, h, c * FN:(c + 1) * FN], ps_o, rden)

        def proj_group(c, gi, drain=None, ps=None):
            m = 4 * c + gi // (H // FN)
            c2 = gi % (H // FN)
            if ps is None:
                ps = psh_tile()
            for k in range(HG):
                nc.tensor.matmul(
                    ps, outt[:, k, m * P:(m + 1) * P],
                    wo_s[:, k, c2 * FN:(c2 + 1) * FN],
                    start=(k == 0), stop=(k == HG - 1))
            yt = ystage.tile([P, FN], BF16, tag="y", name="yt")
            if drain == "scalar":
                nc.scalar.copy(yt, ps)   # ACT is idle during the final drain
            else:
                nc.vector.tensor_copy(yt, ps)
            nc.sync.dma_start(y[m * P:(m + 1) * P, c2 * FN:(c2 + 1) * FN], yt)

        # ---- V-projection interleaved with chunk-0 scores ----------------
        et_c = {}
        pso_c = {}
        for m in range(5):
            v_group(m)
        et_c[0] = s_head(0, 0)
        for m in range(5, 10):
            v_group(m)
        et_c[1] = s_head(0, 1)
        for m in range(10, 14):
            v_group(m)
        et_c[2] = s_head(0, 2)
        for m in range(14, 16):
            v_group(m)

        xtw.release()   # right stack empty: scratch + wo fit on the left
        sc_pool = ctx.enter_context(tc.tile_pool(name="scr", bufs=3, side="left"))
        wo_pool = ctx.enter_context(tc.tile_pool(name="wo", bufs=1, side="left"))
        wo_s = wo_pool.tile([P, HG, H], BF16)
        nc.sync.dma_start(wo_s, wo.rearrange("(a p) n -> p a n", p=P))

        # ---- chunk 0 (no projection fill available yet) ------------------
        sc_c = {0: tree_head(et_c[0]), 1: tree_head(et_c[1])}
        pso_c[0] = av_head(0, 0, et_c[0])
        om_head(0, 0, sc_c[0], pso_c[0])
        sc_c[2] = tree_head(et_c[2])
        et_c[3] = s_head(0, 3)
        pso_c[1] = av_head(0, 1, et_c[1])
        om_head(0, 1, sc_c[1], pso_c[1])
        pso_c[2] = av_head(0, 2, et_c[2])
        om_head(0, 2, sc_c[2], pso_c[2])
        pso_c[3] = av_head(0, 3, et_c[3])
        sc_c[3] = tree_head(et_c[3])
        carry = (0, 3, sc_c[3], pso_c[3])   # OM deferred into the next chunk

        # ---- chunks 1..3: scores/AV with proj(c-1) as PE fill ------------
        for c in range(1, SQ):
            et_c = {}
            pso_c = {}
            sc_c = {}
            et_c[0] = s_head(c, 0)
            om_head(*carry)
            for gi in range(8):
                proj_group(c - 1, gi)
            sc_c[0] = tree_head(et_c[0])
            et_c[1] = s_head(c, 1)
            for gi in range(8, 16):
                proj_group(c - 1, gi)
            sc_c[1] = tree_head(et_c[1])
            et_c[2] = s_head(c, 2)
            pso_c[0] = av_head(c, 0, et_c[0])
            om_head(c, 0, sc_c[0], pso_c[0])
            sc_c[2] = tree_head(et_c[2])
            et_c[3] = s_head(c, 3)
            pso_c[1] = av_head(c, 1, et_c[1])
            om_head(c, 1, sc_c[1], pso_c[1])
            pso_c[2] = av_head(c, 2, et_c[2])
            om_head(c, 2, sc_c[2], pso_c[2])
            pso_c[3] = av_head(c, 3, et_c[3])
            sc_c[3] = tree_head(et_c[3])
            carry = (c, 3, sc_c[3], pso_c[3])

        # ---- final chunk's projection drains last ------------------------
        # alternate drain engines: ACT still runs the last exps and DVE the
        # last trees when this starts, so a single queue would back up
        om_head(*carry)
        for gi in range(16):
            proj_group(SQ - 1, gi, drain="scalar" if gi % 2 else None)


def _get_nc():
    if "nc" not in _CACHE:
        _CACHE["nc"] = _build_nc()
    return _CACHE["nc"]


def make_in_maps(X, Wq, bq, Wk, bk, Wv, bv, Wo, bo):
    bf16 = ml_dtypes.bfloat16
    scale = 1.0 / math.sqrt(D)
    X = np.asarray(X, dtype=np.float32)
    xt_b = [np.ascontiguousarray(X[b].T).astype(bf16) for b in range(B)]
    Wq = np.asarray(Wq, dtype=np.float32) * scale
    Wk = np.asarray(Wk, dtype=np.float32)
    Wv = np.asarray(Wv, dtype=np.float32)
    Wo = np.asarray(Wo, dtype=np.float32)
    bq = np.asarray(bq, dtype=np.float32) * scale
    bk = np.asarray(bk, dtype=np.float32)
    bv = np.asarray(bv, dtype=np.float32)
    in_maps = []
    for c in range(NC):
        b, g = divmod(c, GROUPS)
        sl = slice(g * CW, (g + 1) * CW)
        # per-partition bias layout: bqk[p, m] = bq[m*128+p], cols 4..7 = bk
        bqk = np.concatenate(
            [bq[sl].reshape(HG, P).T, bk[sl].reshape(HG, P).T], axis=1)
        in_maps.append({
            "xt": xt_b[b],
            "wq": np.ascontiguousarray(Wq[:, sl]).astype(bf16),
            "wk": np.ascontiguousarray(Wk[:, sl]).astype(bf16),
            "wv": np.ascontiguousarray(Wv[:, sl]).astype(bf16),
            "wo": np.ascontiguousarray(Wo[sl, :]).astype(bf16),
            "bqk": np.ascontiguousarray(bqk, dtype=np.float32),
            "bv": np.ascontiguousarray(bv[sl]),
        })
    return in_maps


def gather_output(results, bo):
    bo = np.asarray(bo, dtype=np.float32)
    out = np.empty((B, S, H), np.float32)
    for b in range(B):
        acc = results[b * GROUPS]["y"].astype(np.float32)
        for g in range(1, GROUPS):
            acc += results[b * GROUPS + g]["y"].astype(np.float32)
        out[b] = acc + bo[None, :]
    return out


def kernel(X, Wq, bq, Wk, bk, Wv, bv, Wo, bo):
    from concourse.bass_utils import run_bass_kernel_spmd

    in_maps = make_in_maps(X, Wq, bq, Wk, bk, Wv, bv, Wo, bo)
    nc = _get_nc()
    res = run_bass_kernel_spmd(nc, in_maps, list(range(NC))).results
    return gather_output(res, bo)
